# revision 22
# baseline (speedup 1.0000x reference)
"""MoE routing kernel (top-2 of 32 experts, dense-mix form) for 8 TRN2 cores.

Math identity: out = sum_e mix_w[:, e] * (x @ W_e) + mix_b @ expert_biases,
where mix_w / mix_b are dense top-2 softmax mixture coefficients. Experts are
sharded 4-per-core; each core computes a partial sum and the host adds the 8
partials.

Schedule notes (v7):
- Router columns are PERMUTED per core (host-side) so this core's 4 experts
  occupy logit columns 0:4 (weights) and 32:36 (biases); top-2+softmax over a
  32-column half is permutation-invariant, so local mix coefficients are read
  straight out of mix_comb — no select matmul.
- Router matmuls stay fp32 end-to-end: top-2 selection is discontinuous and a
  single flipped pick vs the fp32 reference costs ~3e-2 rel error (the tightest
  top2/top3 logit gap in this input set is ~1.3e-3; bf16 anywhere in the router
  path flips it).
- The bias-bank matmul (K=4) runs in bf16 and sits after expert 1, where the
  PE would otherwise wait on weight DMAs.
- Warmup matmuls on a zeroed tile run first so the PE HAM clock gate (cold
  1.2GHz -> warm 2.4GHz after ~3.4us of sustained activity) is released by the
  time real matmuls start.
- Weights stream as 8KB-per-partition chunks through a 4-slot tile pool: each
  chunk's DMA issue self-gates on the matmuls that consumed the slot two
  generations earlier. Keeping every DMA's issue within a few us of its
  transfer slot matters: long-queued DMAs were measured to post their
  completion semaphores ~6us after their bytes landed, while shallow-queue
  DMAs post within ~1-2us.
- Expert 3 is laid out column-half-major in DRAM so each half streams as
  contiguous rows (strided half-column DMAs cost 2.8-6.5us of descriptor
  generation each); h0 finishes first so its output DMA overlaps h1's stream,
  and the last h1 chunk is small so the final matmuls unblock early.
"""

import sys

if "/opt/trn_rl_repo" not in sys.path:
    sys.path.insert(0, "/opt/trn_rl_repo")

from contextlib import ExitStack

import ml_dtypes
import numpy as np

import concourse.bacc as bacc
import concourse.tile as tile
from concourse import mybir
from concourse.bass_utils import run_bass_kernel_spmd
from concourse.masks import make_identity

B = 128        # batch
D = 1024       # in = out features
E = 32         # experts
NCORES = 8
EPC = E // NCORES   # experts per core
KT = D // 128       # k-tiles of 128 along contraction dim
HD = 512            # psum-bank-sized output chunk
WCH = 4             # k-tiles per weight DMA chunk (experts 0-2)
N_WARM = 10         # PE warmup matmuls (HAM clock-gate release)

F32 = mybir.dt.float32
BF16 = mybir.dt.bfloat16
ALU = mybir.AluOpType
ACTF = mybir.ActivationFunctionType


def _ctile(pool, name, shape, dtype):
    # unique tag => dedicated slot, never rotated/reused
    return pool.tile(shape, dtype, name=name, tag=name)


def build_program(reps=1):
    nc = bacc.Bacc("TRN2")

    # x and the two routers packed in one tensor: cols 0:B are x^T k-tiles,
    # cols B: are [router_w | bias_router_w] (columns permuted per core)
    xrw_d = nc.dram_tensor("xrw", [128, KT, B + 2 * E], F32, kind="ExternalInput")
    wloc_d = nc.dram_tensor("wloc", [EPC - 1, 128, KT, D], BF16, kind="ExternalInput")
    # expert 3 column-half-major: [128, half, kt, 512]
    wl3_d = nc.dram_tensor("wl3", [128, 2, KT, HD], BF16, kind="ExternalInput")
    bscl_d = nc.dram_tensor("bscl", [EPC, D], BF16, kind="ExternalInput")
    out_d = nc.dram_tensor("out", [B, D], BF16, kind="ExternalOutput")

    with ExitStack() as ctx:
        tc = ctx.enter_context(tile.TileContext(nc))
        const = ctx.enter_context(tc.tile_pool(name="const", bufs=1))
        wpool = ctx.enter_context(tc.tile_pool(name="wch", bufs=4))
        pp_a = ctx.enter_context(tc.tile_pool(name="pa", bufs=1, space="PSUM"))
        pp_t = ctx.enter_context(tc.tile_pool(name="pt", bufs=1, space="PSUM"))
        pp_e = ctx.enter_context(tc.tile_pool(name="pe", bufs=3, space="PSUM"))

        junk = _ctile(const, "junk", [128, HD], BF16)
        nc.gpsimd.memset(junk[:], 0.0)
        ident = _ctile(const, "ident", [128, 128], F32)
        make_identity(nc, ident[:])

        # PE warmup: data-independent matmuls release the HAM clock gate
        # before real matmuls need it
        wps = pp_e.tile([B, 2, HD], F32, name="pe")
        for _ in range(N_WARM):
            nc.tensor.matmul(wps[:, 0, :], junk[:, 0:128], junk[:], start=True, stop=True)

        for _ in range(reps):
            xrw = _ctile(const, "xrw", [128, KT, B + 2 * E], F32)
            bscl = _ctile(const, "bscl", [EPC, D], BF16)

            # ---- DMA schedule: single sync ring; chunk tiles allocated
            # right before their dma_start so pool-slot rotation gates each
            # issue on the consumption of the slot 4 chunks earlier ----
            nc.sync.dma_start(xrw[:], xrw_d[:])
            wc = []
            wc.append(wpool.tile([128, WCH, D], BF16, name="w"))
            nc.sync.dma_start(wc[0][:], wloc_d[0, :, 0:WCH, :])
            nc.sync.dma_start(bscl[:], bscl_d[:])
            for ch in range(1, 6):
                e, half = divmod(ch, 2)
                t = wpool.tile([128, WCH, D], BF16, name="w")
                wc.append(t)
                nc.sync.dma_start(
                    t[:], wloc_d[e, :, half * WCH:(half + 1) * WCH, :]
                )
            w3h = []
            for h in range(2):
                t = wpool.tile([128, KT, HD], BF16, name="w")
                w3h.append(t)
                if h == 0:
                    nc.sync.dma_start(t[:], wl3_d[:, 0, :, :])
                else:
                    # small final chunk => the last matmuls unblock early
                    nc.sync.dma_start(t[:, 0:6, :], wl3_d[:, 1, 0:6, :])
                    nc.sync.dma_start(t[:, 6:8, :], wl3_d[:, 1, 6:8, :])

            # ---- on-chip bf16 cast of x, on DVE ----
            xtb = _ctile(const, "xtb", [128, KT, B], BF16)
            for j in range(0, KT, 2):
                nc.vector.tensor_scalar_add(
                    xtb[:, j:j + 2, :], xrw[:, j:j + 2, 0:B], 0.0
                )

            # ---- router logits: [B, 64] = x @ [router_w | bias_router_w] ----
            pl = pp_a.tile([B, 2 * E], F32, name="pa")
            for k in range(KT):
                nc.tensor.matmul(
                    pl[:], xrw[:, k, 0:B], xrw[:, k, B:B + 2 * E],
                    start=(k == 0), stop=(k == KT - 1),
                )
            logits = _ctile(const, "logits", [B, 2 * E], F32)
            nc.scalar.copy(logits[:], pl[:])

            # experts 0/1 matmuls issue now: PE is in-order, so queueing them
            # ahead of the mix/bias chain lets PE crunch while DVE routes
            pes = []
            for e in range(2):
                pe = pp_e.tile([B, 2, HD], F32, name="pe")
                pes.append(pe)
                for k in range(KT):
                    for c in range(2):
                        nc.tensor.matmul(
                            pe[:, c, :], xtb[:, k, :],
                            wc[2 * e + k // WCH][:, k % WCH, c * HD:(c + 1) * HD],
                            start=(k == 0), stop=(k == KT - 1),
                        )

            # ---- top-2 + softmax per half -> dense mix coeffs [B, 64] ----
            # columns are host-permuted so local experts sit at 0:4 / 32:36
            mix_comb = _ctile(const, "mix_comb", [B, 2 * E], F32)
            for h in range(2):
                lh = logits[:, h * E:(h + 1) * E]
                mx1 = _ctile(const, f"mx1_{h}", [B, 1], F32)
                nc.vector.tensor_reduce(mx1[:], lh, axis=mybir.AxisListType.X, op=ALU.max)
                m1 = _ctile(const, f"m1_{h}", [B, E], F32)
                nc.vector.tensor_scalar(m1[:], lh, mx1[:], None, op0=ALU.is_ge)
                msk = _ctile(const, f"msk_{h}", [B, E], F32)
                nc.vector.scalar_tensor_tensor(
                    msk[:], m1[:], -1e30, lh, op0=ALU.mult, op1=ALU.add
                )
                mx2 = _ctile(const, f"mx2_{h}", [B, 1], F32)
                nc.vector.tensor_reduce(mx2[:], msk[:], axis=mybir.AxisListType.X, op=ALU.max)
                m2 = _ctile(const, f"m2_{h}", [B, E], F32)
                nc.vector.tensor_scalar(m2[:], msk[:], mx2[:], None, op0=ALU.is_ge)
                dgap = _ctile(const, f"dgap_{h}", [B, 1], F32)
                nc.vector.tensor_sub(dgap[:], mx2[:], mx1[:])
                ed = _ctile(const, f"ed_{h}", [B, 1], F32)
                nc.scalar.activation(ed[:], dgap[:], ACTF.Exp)
                den = _ctile(const, f"den_{h}", [B, 1], F32)
                nc.vector.tensor_scalar_add(den[:], ed[:], 1.0)
                p1 = _ctile(const, f"p1_{h}", [B, 1], F32)
                nc.vector.reciprocal(p1[:], den[:])
                p2 = _ctile(const, f"p2_{h}", [B, 1], F32)
                nc.vector.tensor_mul(p2[:], ed[:], p1[:])
                t2 = _ctile(const, f"t2_{h}", [B, E], F32)
                nc.vector.tensor_scalar_mul(t2[:], m2[:], p2[:])
                nc.vector.scalar_tensor_tensor(
                    mix_comb[:, h * E:(h + 1) * E], m1[:], p1[:], t2[:],
                    op0=ALU.mult, op1=ALU.add,
                )

            # ---- local bias term: mixb @ bscl via [4, B] transpose, bf16.
            # Emitted after experts 0/1 so PE reaches it while waiting on
            # later weight chunks ----
            ptb = pp_t.tile([2 * E, B], F32, name="pt")
            nc.tensor.transpose(
                ptb[0:EPC, :], mix_comb[:, E:E + EPC], ident[:]
            )
            mixbT = _ctile(const, "mixbT", [EPC, B], BF16)
            nc.scalar.copy(mixbT[:], ptb[0:EPC, :])
            pb = pp_e.tile([B, 2, HD], F32, name="pe")
            for c in range(2):
                nc.tensor.matmul(
                    pb[:, c, :], mixbT[:], bscl[:, c * HD:(c + 1) * HD],
                    start=True, stop=True,
                )
            bias_sb = _ctile(const, "bias_sb", [B, D], F32)
            for c in range(2):
                nc.scalar.copy(bias_sb[:, c * HD:(c + 1) * HD], pb[:, c, :])

            # ---- STT chain: acc_e = (x @ W_e) * mix_comb[:, e] + acc_{e-1}
            prev = bias_sb
            for e in range(2):
                acc = _ctile(const, f"acc{e}", [B, D], F32)
                for c in range(2):
                    nc.vector.scalar_tensor_tensor(
                        acc[:, c * HD:(c + 1) * HD], pes[e][:, c, :],
                        mix_comb[:, e:e + 1],
                        prev[:, c * HD:(c + 1) * HD], op0=ALU.mult, op1=ALU.add,
                    )
                prev = acc

            # expert 2
            pe2 = pp_e.tile([B, 2, HD], F32, name="pe")
            for k in range(KT):
                for c in range(2):
                    nc.tensor.matmul(
                        pe2[:, c, :], xtb[:, k, :],
                        wc[4 + k // WCH][:, k % WCH, c * HD:(c + 1) * HD],
                        start=(k == 0), stop=(k == KT - 1),
                    )
            acc2 = _ctile(const, "acc2", [B, D], F32)
            for c in range(2):
                nc.vector.scalar_tensor_tensor(
                    acc2[:, c * HD:(c + 1) * HD], pe2[:, c, :],
                    mix_comb[:, 2:3],
                    prev[:, c * HD:(c + 1) * HD], op0=ALU.mult, op1=ALU.add,
                )
            prev = acc2

            # last expert: per-half compute -> evict -> output DMA
            el = EPC - 1
            pel = pp_e.tile([B, 2, HD], F32, name="pe")
            # bf16 final accumulator: host sums the 8 partials in f64 anyway
            accl = _ctile(const, f"acc{el}", [B, D], BF16)
            for h in range(2):
                hs, he = h * HD, (h + 1) * HD
                for k in range(KT):
                    nc.tensor.matmul(
                        pel[:, h, :], xtb[:, k, :],
                        w3h[h][:, k, :],
                        start=(k == 0), stop=(k == KT - 1),
                    )
                nc.vector.scalar_tensor_tensor(
                    accl[:, hs:he], pel[:, h, :], mix_comb[:, el:el + 1],
                    prev[:, hs:he], op0=ALU.mult, op1=ALU.add,
                )
                # h0 rides the (otherwise idle) scalar ring; h1 rides sync
                eng = nc.sync if h == 1 else nc.scalar
                eng.dma_start(out_d[:, hs:he], accl[:, hs:he])

    nc.finalize()
    return nc


def make_input_maps(x, router_w, bias_router_w, expert_weights, expert_biases):
    xt = x.T.reshape(KT, 128, B).transpose(1, 0, 2)

    in_maps = []
    for c in range(NCORES):
        # permute router columns so this core's experts are first: top-2 +
        # softmax over each 32-col half is permutation-invariant, and the
        # local mix coefficients land in mix_comb[:, 0:4] / [:, 32:36]
        perm = list(range(c * EPC, (c + 1) * EPC)) + [
            j for j in range(E) if not (c * EPC <= j < (c + 1) * EPC)
        ]
        rw2 = (
            np.concatenate([router_w[:, perm], bias_router_w[:, perm]], axis=1)
            .reshape(KT, 128, 2 * E)
            .transpose(1, 0, 2)
        )
        xrw = np.ascontiguousarray(
            np.concatenate([xt, rw2], axis=2), dtype=np.float32
        )
        wl_all = (
            expert_weights[c * EPC:(c + 1) * EPC]
            .reshape(EPC, KT, 128, D)
            .transpose(0, 2, 1, 3)
        )  # [EPC, 128, KT, D]
        wl = np.ascontiguousarray(wl_all[:EPC - 1]).astype(ml_dtypes.bfloat16)
        # expert 3 column-half-major: [128, half, KT, HD]
        wl3 = (
            wl_all[EPC - 1]
            .reshape(128, KT, 2, HD)
            .transpose(0, 2, 1, 3)
        )
        wl3 = np.ascontiguousarray(wl3).astype(ml_dtypes.bfloat16)
        bscl = np.ascontiguousarray(
            expert_biases[c * EPC:(c + 1) * EPC]
        ).astype(ml_dtypes.bfloat16)
        in_maps.append(dict(xrw=xrw, wloc=wl, wl3=wl3, bscl=bscl))
    return in_maps


def kernel(x, router_w, bias_router_w, expert_weights, expert_biases, **bench_kwargs):
    in_maps = make_input_maps(x, router_w, bias_router_w, expert_weights, expert_biases)
    nc = build_program()
    res = run_bass_kernel_spmd(nc, in_maps, list(range(NCORES)), **bench_kwargs)
    out = np.zeros((B, D), dtype=np.float64)
    for r in res.results:
        out += r["out"].astype(np.float64)
    final = out.astype(np.float32)
    if bench_kwargs:
        kernel.last_result = res
    return final


# revision 23
# speedup vs baseline: 1.1055x; 1.1055x over previous
"""MoE routing kernel (top-2 of 32 experts, dense-mix form) for 8 TRN2 cores.

Math identity: out = sum_e mix_w[:, e] * (x @ W_e) + mix_b @ expert_biases,
where mix_w / mix_b are dense top-2 softmax mixture coefficients. Experts are
sharded 4-per-core; each core computes a partial sum and the host adds the 8
partials.

Schedule notes (v7):
- Router columns are PERMUTED per core (host-side) so this core's 4 experts
  occupy logit columns 0:4 (weights) and 32:36 (biases); top-2+softmax over a
  32-column half is permutation-invariant, so local mix coefficients are read
  straight out of mix_comb — no select matmul.
- Router matmuls stay fp32 end-to-end: top-2 selection is discontinuous and a
  single flipped pick vs the fp32 reference costs ~3e-2 rel error (the tightest
  top2/top3 logit gap in this input set is ~1.3e-3; bf16 anywhere in the router
  path flips it).
- The bias-bank matmul (K=4) runs in bf16 and sits after expert 1, where the
  PE would otherwise wait on weight DMAs.
- Warmup matmuls on a zeroed tile run first so the PE HAM clock gate (cold
  1.2GHz -> warm 2.4GHz after ~3.4us of sustained activity) is released by the
  time real matmuls start.
- Weights stream as 8KB-per-partition chunks through a 4-slot tile pool: each
  chunk's DMA issue self-gates on the matmuls that consumed the slot two
  generations earlier. Keeping every DMA's issue within a few us of its
  transfer slot matters: long-queued DMAs were measured to post their
  completion semaphores ~6us after their bytes landed, while shallow-queue
  DMAs post within ~1-2us.
- Expert 3 is laid out column-half-major in DRAM so each half streams as
  contiguous rows (strided half-column DMAs cost 2.8-6.5us of descriptor
  generation each); h0 finishes first so its output DMA overlaps h1's stream,
  and the last h1 chunk is small so the final matmuls unblock early.
"""

import sys

if "/opt/trn_rl_repo" not in sys.path:
    sys.path.insert(0, "/opt/trn_rl_repo")

from contextlib import ExitStack

import ml_dtypes
import numpy as np

import concourse.bacc as bacc
import concourse.tile as tile
from concourse import mybir
from concourse.bass_utils import run_bass_kernel_spmd
from concourse.masks import make_identity

B = 128        # batch
D = 1024       # in = out features
E = 32         # experts
NCORES = 8
EPC = E // NCORES   # experts per core
KT = D // 128       # k-tiles of 128 along contraction dim
HD = 512            # psum-bank-sized output chunk
WCH = 4             # k-tiles per weight DMA chunk (experts 0-2)
N_WARM = 10         # PE warmup matmuls (HAM clock-gate release)

F32 = mybir.dt.float32
BF16 = mybir.dt.bfloat16
ALU = mybir.AluOpType
ACTF = mybir.ActivationFunctionType


def _ctile(pool, name, shape, dtype):
    # unique tag => dedicated slot, never rotated/reused
    return pool.tile(shape, dtype, name=name, tag=name)


def build_program(reps=1):
    nc = bacc.Bacc("TRN2")

    # x and the two routers packed in one tensor: cols 0:B are x^T k-tiles,
    # cols B: are [router_w | bias_router_w] (columns permuted per core)
    xrw_d = nc.dram_tensor("xrw", [128, KT, B + 2 * E], F32, kind="ExternalInput")
    wloc_d = nc.dram_tensor("wloc", [EPC - 1, 128, KT, D], BF16, kind="ExternalInput")
    # expert 3 column-half-major: [128, half, kt, 512]
    wl3_d = nc.dram_tensor("wl3", [128, 2, KT, HD], BF16, kind="ExternalInput")
    bscl_d = nc.dram_tensor("bscl", [EPC, D], BF16, kind="ExternalInput")
    out_d = nc.dram_tensor("out", [B, D], BF16, kind="ExternalOutput")

    with ExitStack() as ctx:
        tc = ctx.enter_context(tile.TileContext(nc))
        const = ctx.enter_context(tc.tile_pool(name="const", bufs=1))
        wpool = ctx.enter_context(tc.tile_pool(name="wch", bufs=4))
        pp_a = ctx.enter_context(tc.tile_pool(name="pa", bufs=1, space="PSUM"))
        pp_t = ctx.enter_context(tc.tile_pool(name="pt", bufs=1, space="PSUM"))
        pp_e = ctx.enter_context(tc.tile_pool(name="pe", bufs=3, space="PSUM"))

        junk = _ctile(const, "junk", [128, HD], BF16)
        nc.gpsimd.memset(junk[:], 0.0)
        ident = _ctile(const, "ident", [128, 128], F32)
        make_identity(nc, ident[:])

        # PE warmup: data-independent matmuls release the HAM clock gate
        # before real matmuls need it
        wps = pp_e.tile([B, 2, HD], F32, name="pe")
        for _ in range(N_WARM):
            nc.tensor.matmul(wps[:, 0, :], junk[:, 0:128], junk[:], start=True, stop=True)

        for _ in range(reps):
            xrw = _ctile(const, "xrw", [128, KT, B + 2 * E], F32)
            bscl = _ctile(const, "bscl", [EPC, D], BF16)

            # ---- DMA schedule: single sync ring; chunk tiles allocated
            # right before their dma_start so pool-slot rotation gates each
            # issue on the consumption of the slot 4 chunks earlier ----
            nc.sync.dma_start(xrw[:], xrw_d[:])
            wc = []
            wc.append(wpool.tile([128, WCH, D], BF16, name="w"))
            nc.sync.dma_start(wc[0][:], wloc_d[0, :, 0:WCH, :])
            nc.sync.dma_start(bscl[:], bscl_d[:])
            for ch in range(1, 6):
                e, half = divmod(ch, 2)
                t = wpool.tile([128, WCH, D], BF16, name="w")
                wc.append(t)
                nc.sync.dma_start(
                    t[:], wloc_d[e, :, half * WCH:(half + 1) * WCH, :]
                )
            w3h = []
            for h in range(2):
                t = wpool.tile([128, KT, HD], BF16, name="w")
                w3h.append(t)
                if h == 0:
                    nc.sync.dma_start(t[:], wl3_d[:, 0, :, :])
                else:
                    # small final chunk => the last matmuls unblock early
                    nc.sync.dma_start(t[:, 0:6, :], wl3_d[:, 1, 0:6, :])
                    nc.sync.dma_start(t[:, 6:8, :], wl3_d[:, 1, 6:8, :])

            # ---- on-chip bf16 cast of x, on DVE ----
            xtb = _ctile(const, "xtb", [128, KT, B], BF16)
            for j in range(0, KT, 2):
                nc.vector.tensor_scalar_add(
                    xtb[:, j:j + 2, :], xrw[:, j:j + 2, 0:B], 0.0
                )

            # ---- router logits: [B, 64] = x @ [router_w | bias_router_w] ----
            pl = pp_a.tile([B, 2 * E], F32, name="pa")
            for k in range(KT):
                nc.tensor.matmul(
                    pl[:], xrw[:, k, 0:B], xrw[:, k, B:B + 2 * E],
                    start=(k == 0), stop=(k == KT - 1),
                )
            logits = _ctile(const, "logits", [B, 2 * E], F32)
            nc.scalar.copy(logits[:], pl[:])

            # experts 0/1 matmuls issue now: PE is in-order, so queueing them
            # ahead of the mix/bias chain lets PE crunch while DVE routes
            pes = []
            for e in range(2):
                pe = pp_e.tile([B, 2, HD], F32, name="pe")
                pes.append(pe)
                for k in range(KT):
                    for c in range(2):
                        nc.tensor.matmul(
                            pe[:, c, :], xtb[:, k, :],
                            wc[2 * e + k // WCH][:, k % WCH, c * HD:(c + 1) * HD],
                            start=(k == 0), stop=(k == KT - 1),
                        )

            # ---- top-2 + softmax per half -> dense mix coeffs [B, 64] ----
            # columns are host-permuted so local experts sit at 0:4 / 32:36
            mix_comb = _ctile(const, "mix_comb", [B, 2 * E], F32)
            for h in range(2):
                lh = logits[:, h * E:(h + 1) * E]
                mx1 = _ctile(const, f"mx1_{h}", [B, 1], F32)
                nc.vector.tensor_reduce(mx1[:], lh, axis=mybir.AxisListType.X, op=ALU.max)
                m1 = _ctile(const, f"m1_{h}", [B, E], F32)
                nc.vector.tensor_scalar(m1[:], lh, mx1[:], None, op0=ALU.is_ge)
                msk = _ctile(const, f"msk_{h}", [B, E], F32)
                nc.vector.scalar_tensor_tensor(
                    msk[:], m1[:], -1e30, lh, op0=ALU.mult, op1=ALU.add
                )
                mx2 = _ctile(const, f"mx2_{h}", [B, 1], F32)
                nc.vector.tensor_reduce(mx2[:], msk[:], axis=mybir.AxisListType.X, op=ALU.max)
                m2 = _ctile(const, f"m2_{h}", [B, E], F32)
                nc.vector.tensor_scalar(m2[:], msk[:], mx2[:], None, op0=ALU.is_ge)
                dgap = _ctile(const, f"dgap_{h}", [B, 1], F32)
                nc.vector.tensor_sub(dgap[:], mx2[:], mx1[:])
                ed = _ctile(const, f"ed_{h}", [B, 1], F32)
                nc.scalar.activation(ed[:], dgap[:], ACTF.Exp)
                den = _ctile(const, f"den_{h}", [B, 1], F32)
                nc.vector.tensor_scalar_add(den[:], ed[:], 1.0)
                p1 = _ctile(const, f"p1_{h}", [B, 1], F32)
                nc.vector.reciprocal(p1[:], den[:])
                p2 = _ctile(const, f"p2_{h}", [B, 1], F32)
                nc.vector.tensor_mul(p2[:], ed[:], p1[:])
                t2 = _ctile(const, f"t2_{h}", [B, E], F32)
                nc.vector.tensor_scalar_mul(t2[:], m2[:], p2[:])
                nc.vector.scalar_tensor_tensor(
                    mix_comb[:, h * E:(h + 1) * E], m1[:], p1[:], t2[:],
                    op0=ALU.mult, op1=ALU.add,
                )

            # ---- local bias term: mixb @ bscl via [4, B] transpose, bf16.
            # Emitted after experts 0/1 so PE reaches it while waiting on
            # later weight chunks ----
            ptb = pp_t.tile([2 * E, B], F32, name="pt")
            nc.tensor.transpose(
                ptb[0:EPC, :], mix_comb[:, E:E + EPC], ident[:]
            )
            mixbT = _ctile(const, "mixbT", [EPC, B], BF16)
            nc.scalar.copy(mixbT[:], ptb[0:EPC, :])
            pb = pp_e.tile([B, 2, HD], F32, name="pe")
            for c in range(2):
                nc.tensor.matmul(
                    pb[:, c, :], mixbT[:], bscl[:, c * HD:(c + 1) * HD],
                    start=True, stop=True,
                )
            bias_sb = _ctile(const, "bias_sb", [B, D], F32)
            for c in range(2):
                nc.scalar.copy(bias_sb[:, c * HD:(c + 1) * HD], pb[:, c, :])

            # ---- STT chain: acc_e = (x @ W_e) * mix_comb[:, e] + acc_{e-1}
            prev = bias_sb
            for e in range(2):
                acc = _ctile(const, f"acc{e}", [B, D], F32)
                for c in range(2):
                    nc.vector.scalar_tensor_tensor(
                        acc[:, c * HD:(c + 1) * HD], pes[e][:, c, :],
                        mix_comb[:, e:e + 1],
                        prev[:, c * HD:(c + 1) * HD], op0=ALU.mult, op1=ALU.add,
                    )
                prev = acc

            # expert 2
            pe2 = pp_e.tile([B, 2, HD], F32, name="pe")
            for k in range(KT):
                for c in range(2):
                    nc.tensor.matmul(
                        pe2[:, c, :], xtb[:, k, :],
                        wc[4 + k // WCH][:, k % WCH, c * HD:(c + 1) * HD],
                        start=(k == 0), stop=(k == KT - 1),
                    )
            acc2 = _ctile(const, "acc2", [B, D], F32)
            for c in range(2):
                nc.vector.scalar_tensor_tensor(
                    acc2[:, c * HD:(c + 1) * HD], pe2[:, c, :],
                    mix_comb[:, 2:3],
                    prev[:, c * HD:(c + 1) * HD], op0=ALU.mult, op1=ALU.add,
                )
            prev = acc2

            # last expert: per-half compute -> evict -> output DMA
            el = EPC - 1
            pel = pp_e.tile([B, 2, HD], F32, name="pe")
            # bf16 final accumulator: host sums the 8 partials in f64 anyway
            accl = _ctile(const, f"acc{el}", [B, D], BF16)
            for h in range(2):
                hs, he = h * HD, (h + 1) * HD
                for k in range(KT):
                    nc.tensor.matmul(
                        pel[:, h, :], xtb[:, k, :],
                        w3h[h][:, k, :],
                        start=(k == 0), stop=(k == KT - 1),
                    )
                nc.vector.scalar_tensor_tensor(
                    accl[:, hs:he], pel[:, h, :], mix_comb[:, el:el + 1],
                    prev[:, hs:he], op0=ALU.mult, op1=ALU.add,
                )
                # both halves ride the (otherwise idle) scalar ring so their
                # completions don't queue behind the sync ring's receipts
                nc.scalar.dma_start(out_d[:, hs:he], accl[:, hs:he])

    nc.finalize()
    return nc


def make_input_maps(x, router_w, bias_router_w, expert_weights, expert_biases):
    xt = x.T.reshape(KT, 128, B).transpose(1, 0, 2)

    in_maps = []
    for c in range(NCORES):
        # permute router columns so this core's experts are first: top-2 +
        # softmax over each 32-col half is permutation-invariant, and the
        # local mix coefficients land in mix_comb[:, 0:4] / [:, 32:36]
        perm = list(range(c * EPC, (c + 1) * EPC)) + [
            j for j in range(E) if not (c * EPC <= j < (c + 1) * EPC)
        ]
        rw2 = (
            np.concatenate([router_w[:, perm], bias_router_w[:, perm]], axis=1)
            .reshape(KT, 128, 2 * E)
            .transpose(1, 0, 2)
        )
        xrw = np.ascontiguousarray(
            np.concatenate([xt, rw2], axis=2), dtype=np.float32
        )
        wl_all = (
            expert_weights[c * EPC:(c + 1) * EPC]
            .reshape(EPC, KT, 128, D)
            .transpose(0, 2, 1, 3)
        )  # [EPC, 128, KT, D]
        wl = np.ascontiguousarray(wl_all[:EPC - 1]).astype(ml_dtypes.bfloat16)
        # expert 3 column-half-major: [128, half, KT, HD]
        wl3 = (
            wl_all[EPC - 1]
            .reshape(128, KT, 2, HD)
            .transpose(0, 2, 1, 3)
        )
        wl3 = np.ascontiguousarray(wl3).astype(ml_dtypes.bfloat16)
        bscl = np.ascontiguousarray(
            expert_biases[c * EPC:(c + 1) * EPC]
        ).astype(ml_dtypes.bfloat16)
        in_maps.append(dict(xrw=xrw, wloc=wl, wl3=wl3, bscl=bscl))
    return in_maps


def kernel(x, router_w, bias_router_w, expert_weights, expert_biases, **bench_kwargs):
    in_maps = make_input_maps(x, router_w, bias_router_w, expert_weights, expert_biases)
    nc = build_program()
    res = run_bass_kernel_spmd(nc, in_maps, list(range(NCORES)), **bench_kwargs)
    out = np.zeros((B, D), dtype=np.float64)
    for r in res.results:
        out += r["out"].astype(np.float64)
    final = out.astype(np.float32)
    if bench_kwargs:
        kernel.last_result = res
    return final
